# revision 44
# baseline (speedup 1.0000x reference)
"""LoRA attention Bass kernel for 8x Trainium2 NeuronCores.

Sharding (Megatron tensor-parallel over heads):
  - Each of the 8 cores owns 2 heads (128 projection columns).
  - q/k/v projections column-sharded; out projection row-sharded;
    per-core partial outputs are summed on the host.
  - LoRA is merged into the base weights on the host (w_eff = w + a@u*scaling),
    exact up to fp32 rounding; host also casts x/wq/wk/wv to bf16.

Device layout (per core), all matmuls bf16 with fp32 PSUM accumulate:
  Phase 1: qT/kT/vT computed transposed ([proj_col, seq]) from xT tiles at
      full PE rate (N=512 moving dim); one batched DMA issue per s-chunk
      brings all 8 k-chunks. PSUM evictions fused with bias + bf16 cast on
      ACT. v natural layout recovered with PE transpose-mode (identity
      matmul), emitted one chunk late so transposes never head-of-line
      block the PE queue. v slabs are 16B-aligned with a fused ones column
      per head ([0:64]=A, 64=onesA, [72:136]=B, 136=onesB).
  Phase 2: S^T = K @ Q^T with BOTH heads as row-tiled concurrent matmuls
      (K=64 at tile_position (0,0)/(64,0)) into one [128,1024] 2-bank PSUM
      tile; one exp over [128,1024] on ACT into bf16. P@V with
      lhsT=[v | ones] so the softmax denominator falls out of the same
      matmul (row 64). PV matmuls are software-pipelined two t-steps behind
      the scores so they never wait on exp at the head of the PE queue.
  Normalization (off the critical path): pv + denom evicted to SBUF right
      after the t-loop (frees PSUM so one pv buffer suffices), denom row
      lane-shifted to partition 0 by a tiny SBUF->SBUF DMA (the custom DVE
      reciprocal and gpsimd partition_broadcast are wrong on HW at nonzero
      base partition), reciprocal_approx_fast, gpsimd partition_broadcast,
      DVE multiply into a [128,512] norm tile whose head-B half is
      lane-shifted by another local DMA.
  Phase 3: out = norm128 @ Wo as single K=128 matmuls (N=512), deferred one
      outer and interleaved into the next outer's t-loop so the PE never
      stalls on the normalization chain; PSUM evicted by DVE, DMAed to DRAM.

PSUM budget (8 banks): scores 2x2 + pv 2 + outproj 2.
"""

import numpy as np

import concourse.bass as bass
import concourse.mybir as mybir
import concourse.tile as tile
from concourse import bacc
from concourse.bass_utils import run_bass_kernel_spmd
from concourse.masks import make_identity
F32 = mybir.dt.float32
F32R = mybir.dt.float32r
BF16 = mybir.dt.bfloat16
AF = mybir.ActivationFunctionType

N_CORES = 8

# Full-problem dims (hardcoded per spec)
D_MODEL = 1024
N_HEADS = 16
D_K = 64
LORA_R = 8
SCALING = 2.0
B = 4
S = 2048


class Cfg:
    """Kernel build configuration."""

    def __init__(self, b=B, s=S, d=D_MODEL, cpc=128, dk=D_K):
        self.b = b                     # batches
        self.s = s                     # seq per batch
        self.d = d                     # model dim (contraction for projections)
        self.cpc = cpc                 # projection cols per core (2 heads x 64)
        self.dk = dk                   # head dim
        self.seq = b * s               # total rows
        self.nkc = d // 128            # k chunks for projections
        self.sc = 512                  # s-chunk width (free dim of matmuls)
        self.nsc = self.seq // self.sc  # s chunks over the whole input
        self.nt = s // 128             # t chunks per batch
        self.nsb = s // self.sc        # s chunks per batch


def _build_nc(cfg: Cfg, dump: bool = False):
    c = cfg
    nc = bacc.Bacc("TRN2", target_bir_lowering=False, debug=False,
                   num_devices=N_CORES)
    n_tchunks_d = c.seq // 128
    if dump:
        qT_d = nc.dram_tensor("qT_d", [128, c.seq], BF16, kind="ExternalOutput").ap()
        kT_d = nc.dram_tensor("kT_d", [128, c.seq], BF16, kind="ExternalOutput").ap()
        v_d = nc.dram_tensor("v_d", [128, n_tchunks_d * 144], BF16, kind="ExternalOutput").ap()
        s_d = nc.dram_tensor("s_d", [128, 1024], F32, kind="ExternalOutput").ap()
        e_d = nc.dram_tensor("e_d", [128, 1024], BF16, kind="ExternalOutput").ap()
        pv_d = nc.dram_tensor("pv_d", [65, 1024], F32, kind="ExternalOutput").ap()
        bcs_d = nc.dram_tensor("bcs_d", [64, 1024], F32, kind="ExternalOutput").ap()
        nrm_d = nc.dram_tensor("nrm_d", [64, 1024], BF16, kind="ExternalOutput").ap()

    xT = nc.dram_tensor("xT", [c.d, c.seq], BF16, kind="ExternalInput").ap()
    wq = nc.dram_tensor("wq", [c.d, c.cpc], BF16, kind="ExternalInput").ap()
    wk = nc.dram_tensor("wk", [c.d, c.cpc], BF16, kind="ExternalInput").ap()
    wv = nc.dram_tensor("wv", [c.d, c.cpc], BF16, kind="ExternalInput").ap()
    wo = nc.dram_tensor("wo", [c.cpc, c.d], F32, kind="ExternalInput").ap()
    bq = nc.dram_tensor("bq", [c.cpc, 1], F32, kind="ExternalInput").ap()
    bk = nc.dram_tensor("bk", [c.cpc, 1], F32, kind="ExternalInput").ap()
    out = nc.dram_tensor("out", [c.seq, c.d], F32, kind="ExternalOutput").ap()

    dk = c.dk
    n_tchunks = c.seq // 128  # global 128-row seq chunks

    with tile.TileContext(nc) as tc:
        with tc.tile_pool(name="persist", bufs=1) as persist:
            # Persistent SBUF tensors
            qT_sb = persist.tile([128, c.seq], BF16, tag="qT")
            kT_sb = persist.tile([128, c.seq], BF16, tag="kT")
            vT_sb = persist.tile([128, c.seq], BF16, tag="vT")
            # v natural + ones columns, 16B-aligned per-head slabs:
            # [0:64]=headA, 64=onesA, [72:136]=headB, 136=onesB, width 144
            VW = 144
            HB = 72
            v_sb = persist.tile([128, n_tchunks, VW], BF16, tag="v")
            wq_sb = persist.tile([128, c.nkc, c.cpc], BF16, tag="wq")
            wk_sb = persist.tile([128, c.nkc, c.cpc], BF16, tag="wk")
            wv_sb = persist.tile([128, c.nkc, c.cpc], BF16, tag="wv")
            wof_sb = persist.tile([c.cpc, c.d], F32, tag="wof")
            wo_sb = persist.tile([c.cpc, c.d], BF16, tag="wo")
            bq_sb = persist.tile([c.cpc, 1], F32, tag="bq")
            bk_sb = persist.tile([c.cpc, 1], F32, tag="bk")

            nc.sync.dma_start(out=wq_sb[:], in_=wq.rearrange("(kc p) m -> p kc m", p=128))
            nc.sync.dma_start(out=wk_sb[:], in_=wk.rearrange("(kc p) m -> p kc m", p=128))
            nc.sync.dma_start(out=wv_sb[:], in_=wv.rearrange("(kc p) m -> p kc m", p=128))
            nc.sync.dma_start(out=wof_sb[:], in_=wo[:])
            nc.sync.dma_start(out=bq_sb[:], in_=bq[:])
            nc.sync.dma_start(out=bk_sb[:], in_=bk[:])
            nc.vector.tensor_copy(wo_sb[:], wof_sb[:])

            ident_sb = persist.tile([128, 128], BF16, tag="ident")
            make_identity(nc, ident_sb[:])

            # ones columns for the fused softmax denominator
            ones_f32 = persist.tile([128, 1], F32, tag="ones_f32")
            nc.vector.memset(ones_f32[:], 1.0)
            nc.vector.tensor_copy(
                v_sb[:, :, dk:dk + 1],
                ones_f32[:].unsqueeze(1).to_broadcast([128, n_tchunks, 1]))
            nc.vector.tensor_copy(
                v_sb[:, :, HB + dk:HB + dk + 1],
                ones_f32[:].unsqueeze(1).to_broadcast([128, n_tchunks, 1]))


            # ---------------- Phase 1: projections ----------------
            with tc.tile_pool(name="xin", bufs=3) as xpool, \
                 tc.tile_pool(name="p1ps", bufs=2, space="PSUM") as p1ps, \
                 tc.tile_pool(name="tpps", bufs=2, space="PSUM") as tpps:
                for sc_i in range(c.nsc):
                    s0 = sc_i * c.sc
                    # one DMA issue brings all 8 k-chunks for this s-range
                    x_t = xpool.tile([128, c.nkc, c.sc], BF16, tag="x")
                    nc.sync.dma_start(
                        out=x_t[:],
                        in_=xT.rearrange("(kc p) s -> p kc s", p=128)[:, :, s0:s0 + c.sc])
                    q_ps = p1ps.tile([128, c.sc], F32, tag="q")
                    k_ps = p1ps.tile([128, c.sc], F32, tag="k")
                    v_ps = p1ps.tile([128, c.sc], F32, tag="v")
                    for kc in range(c.nkc):
                        st = (kc == 0)
                        sp = (kc == c.nkc - 1)
                        nc.tensor.matmul(q_ps[:], wq_sb[:, kc, :], x_t[:, kc, :],
                                         start=st, stop=sp)
                        nc.tensor.matmul(k_ps[:], wk_sb[:, kc, :], x_t[:, kc, :],
                                         start=st, stop=sp)
                        nc.tensor.matmul(v_ps[:], wv_sb[:, kc, :], x_t[:, kc, :],
                                         start=st, stop=sp)
                    nc.scalar.activation(qT_sb[:, s0:s0 + c.sc], q_ps[:],
                                         AF.Identity, bias=bq_sb[:])
                    nc.scalar.activation(kT_sb[:, s0:s0 + c.sc], k_ps[:],
                                         AF.Identity, bias=bk_sb[:])
                    nc.scalar.activation(vT_sb[:, s0:s0 + c.sc], v_ps[:],
                                         AF.Copy)
                    # recover v natural layout with PE transpose-mode, delayed
                    # one chunk so transposes never head-of-line block the PE
                    # queue waiting on this chunk's ACT eviction
                    for sc_t in ([sc_i - 1] if sc_i > 0 else []) + \
                            ([c.nsc - 1] if sc_i == c.nsc - 1 else []):
                        for j in range(c.sc // 128):
                            tci = sc_t * (c.sc // 128) + j
                            t0 = sc_t * c.sc + j * 128
                            tp_ps = tpps.tile([128, 128], BF16, tag="tp")
                            nc.tensor.transpose(
                                tp_ps[:], vT_sb[:, t0:t0 + 128], ident_sb[:])
                            nc.vector.tensor_copy(
                                v_sb[:, tci, 0:dk], tp_ps[:, 0:dk])
                            nc.vector.tensor_copy(
                                v_sb[:, tci, HB:HB + dk], tp_ps[:, dk:2 * dk])

            if dump:
                nc.sync.dma_start(out=qT_d, in_=qT_sb[:])
                nc.sync.dma_start(out=kT_d, in_=kT_sb[:])
                nc.sync.dma_start(out=v_d.rearrange("p (t c) -> p t c", t=n_tchunks_d), in_=v_sb[:])  # width 144

            # ---------------- Phase 2+3: attention + out-proj ----------------
            # PSUM: scores 2x2 banks, pv 2 banks, o_ps 2 banks = 8 banks.
            # pv is evicted to SBUF right after the t-loop so one pv buffer
            # suffices; the normalization chain then runs entirely on SBUF
            # off the critical path, and the out-projection of outer i is
            # interleaved into outer i+1's t-loop (no PE head-of-line stalls).
            with tc.tile_pool(name="sps", bufs=2, space="PSUM") as spool, \
                 tc.tile_pool(name="pvps", bufs=1, space="PSUM") as pvpool, \
                 tc.tile_pool(name="ops", bufs=2, space="PSUM") as opool, \
                 tc.tile_pool(name="exp", bufs=4) as epool, \
                 tc.tile_pool(name="norm", bufs=2) as npool, \
                 tc.tile_pool(name="pvs", bufs=2) as pvspool, \
                 tc.tile_pool(name="bcs", bufs=2) as bcspool, \
                 tc.tile_pool(name="rec", bufs=2) as rpool, \
                 tc.tile_pool(name="osb", bufs=2) as osbpool:
                ew = 512

                def emit_outproj_chunk(norm128, s0, j):
                    o_t = osbpool.tile([128, c.d], F32, tag="osb",
                                       name=f"o_t_{s0}_{j}")
                    for e in range(c.d // ew):
                        o_ps = opool.tile([128, ew], F32, tag="o")
                        nc.tensor.matmul(
                            o_ps[:],
                            norm128[:, j * 128:(j + 1) * 128],
                            wo_sb[:, e * ew:(e + 1) * ew],
                            start=True, stop=True)
                        nc.vector.tensor_copy(
                            o_t[:, e * ew:(e + 1) * ew], o_ps[:])
                    nc.sync.dma_start(
                        out=out[s0 + j * 128:s0 + (j + 1) * 128, :],
                        in_=o_t[:])

                pending = None
                carry = []

                def emit_pv(pv_ab, bi, t, e_ab):
                    tci = bi * c.nt + t
                    nc.tensor.matmul(
                        pv_ab[:, 0:c.sc], v_sb[:, tci, 0:dk + 1],
                        e_ab[:, 0:c.sc],
                        start=(t == 0), stop=(t == c.nt - 1))
                    nc.tensor.matmul(
                        pv_ab[:, c.sc:2 * c.sc],
                        v_sb[:, tci, HB:HB + dk + 1],
                        e_ab[:, c.sc:2 * c.sc],
                        start=(t == 0), stop=(t == c.nt - 1))

                for b_i in range(c.b):
                    for sb_i in range(c.nsb):
                        s0 = b_i * c.s + sb_i * c.sc
                        pv_ab = pvpool.tile([dk + 1, 2 * c.sc], F32, tag="pv")

                        # PV is emitted two t-steps late so it never waits on
                        # exp at the head of the PE queue; the previous
                        # outer's last two PV pairs and its normalization
                        # chain are carried into t=0/1 here so the exp stream
                        # never stalls at outer boundaries.
                        e_hist = []
                        for t in range(c.nt):
                            t0 = b_i * c.s + t * 128
                            s_ab = spool.tile([128, 2 * c.sc], F32, tag="s")
                            nc.tensor.matmul(
                                s_ab[:, 0:c.sc],
                                kT_sb[0:dk, t0:t0 + 128],
                                qT_sb[0:dk, s0:s0 + c.sc],
                                start=True, stop=True,
                                tile_position=(0, 0))
                            nc.tensor.matmul(
                                s_ab[:, c.sc:2 * c.sc],
                                kT_sb[dk:2 * dk, t0:t0 + 128],
                                qT_sb[dk:2 * dk, s0:s0 + c.sc],
                                start=True, stop=True,
                                tile_position=(64, 0))
                            e_ab = epool.tile([128, 2 * c.sc], BF16, tag="e")
                            nc.scalar.activation(e_ab[:], s_ab[:], AF.Exp,
                                                 scale=1.0 / np.sqrt(dk))
                            if dump and b_i == 0 and sb_i == 0 and t == 0:
                                s_stage = npool.tile([128, 1024], F32, tag="sst")
                                nc.vector.tensor_copy(s_stage[:], s_ab[:])
                                nc.sync.dma_start(out=s_d, in_=s_stage[:])
                                nc.sync.dma_start(out=e_d, in_=e_ab[:])
                            # carried work from the previous outer first:
                            # its chain must read the old pv generation
                            # before this outer's first PV write reuses the
                            # banks
                            if t < len(carry):
                                pending = carry[t]() or pending
                            if len(e_hist) == 2:
                                emit_pv(pv_ab, b_i, t - 2, e_hist.pop(0))
                            e_hist.append(e_ab)
                            # out-projection of the previous outer, spread
                            # through this t-loop (chain long since done)
                            if pending is not None and t >= 8 and t % 2 == 0:
                                emit_outproj_chunk(pending[0], pending[1],
                                                  (t - 8) // 2)
                        carry = []

                        def mk_pv_tail(pv_ab=pv_ab, bi=b_i, t=c.nt - 2,
                                       e_h=e_hist[0]):
                            def f():
                                emit_pv(pv_ab, bi, t, e_h)
                            return f

                        def mk_pv_tail2(pv_ab=pv_ab, bi=b_i, t=c.nt - 1,
                                        e_h=e_hist[1]):
                            def f():
                                emit_pv(pv_ab, bi, t, e_h)
                            return f

                        def mk_chain(pv_ab=pv_ab, bi=b_i, si=sb_i, ss0=s0):
                            def f():
                                # free PSUM fast: evict pv data + denom row
                                den_sb = rpool.tile([dk + 1, 2 * c.sc], F32,
                                                    tag="den")
                                pvs = pvspool.tile([dk, 2 * c.sc], F32,
                                                   tag="pvs")
                                nc.vector.tensor_copy(den_sb[dk:dk + 1, :],
                                                      pv_ab[dk:dk + 1, :])
                                nc.vector.tensor_copy(pvs[:], pv_ab[0:dk, :])
                                # normalization chain, all on SBUF
                                den0 = rpool.tile([1, 2 * c.sc], F32,
                                                  tag="den0")
                                rec0 = rpool.tile([1, 2 * c.sc], F32,
                                                  tag="rec0")
                                # lane-shift the denom row to partition 0
                                # (gpsimd broadcast and the custom recip both
                                # need base 0 on HW)
                                nc.sync.dma_start(out=den0[0:1, :],
                                                  in_=den_sb[dk:dk + 1, :])
                                with nc.allow_low_precision(
                                        reason="softmax recip at ~18 bits"):
                                    nc.vector.reciprocal_approx_fast(
                                        out=rec0[0:1, :], in_=den0[0:1, :])
                                bcs_ab = bcspool.tile([dk, 2 * c.sc], F32,
                                                      tag="bab")
                                nc.gpsimd.partition_broadcast(
                                    bcs_ab[:], rec0[0:1, :], channels=dk)
                                norm128 = npool.tile([128, c.sc], BF16,
                                                     tag="n128")
                                nrm_b = npool.tile([dk, c.sc], BF16,
                                                   tag="nbt")
                                nc.vector.tensor_tensor(
                                    norm128[0:dk, :], pvs[:, 0:c.sc],
                                    bcs_ab[:, 0:c.sc], mybir.AluOpType.mult)
                                nc.vector.tensor_tensor(
                                    nrm_b[:], pvs[:, c.sc:2 * c.sc],
                                    bcs_ab[:, c.sc:2 * c.sc],
                                    mybir.AluOpType.mult)
                                # lane-shift head B into partitions 64..127
                                nc.sync.dma_start(out=norm128[dk:2 * dk, :],
                                                  in_=nrm_b[:])
                                if dump and bi == 0 and si == 0:
                                    nc.sync.dma_start(out=pv_d[0:dk, :],
                                                      in_=pvs[:])
                                    nc.sync.dma_start(out=bcs_d,
                                                      in_=bcs_ab[:])
                                    nc.sync.dma_start(out=nrm_d[:, 0:512],
                                                      in_=norm128[0:dk, :])
                                    nc.sync.dma_start(out=nrm_d[:, 512:1024],
                                                      in_=nrm_b[:])
                                return (norm128, ss0)
                            return f

                        carry = [mk_pv_tail(), mk_pv_tail2(), mk_chain()]
                for fn in carry:
                    pending = fn() or pending
                for j in range(c.sc // 128):
                    emit_outproj_chunk(pending[0], pending[1], j)

    nc.compile()
    return nc


_NC_CACHE = {}


def get_nc(cfg: Cfg | None = None):
    cfg = cfg or Cfg()
    key = (cfg.b, cfg.s, cfg.d, cfg.cpc, cfg.dk)
    if key not in _NC_CACHE:
        _NC_CACHE[key] = _build_nc(cfg)
    return _NC_CACHE[key]


def kernel(x, w_q, b_q, w_k, b_k, w_v, b_v, w_o, b_o,
           a_q, u_q, a_k, u_k, a_v, u_v):
    cfg = Cfg()
    c = cfg
    x = np.asarray(x, np.float32)
    w_q = np.asarray(w_q, np.float32)
    w_k = np.asarray(w_k, np.float32)
    w_v = np.asarray(w_v, np.float32)
    w_o = np.asarray(w_o, np.float32)
    b_q = np.asarray(b_q, np.float32)
    b_k = np.asarray(b_k, np.float32)
    b_v = np.asarray(b_v, np.float32)
    b_o = np.asarray(b_o, np.float32)

    def merge(w, a, u):
        return (w.astype(np.float64)
                + (np.asarray(a, np.float64) @ np.asarray(u, np.float64))
                * SCALING).astype(np.float32)

    wq_eff = merge(w_q, a_q, u_q)
    wk_eff = merge(w_k, a_k, u_k)
    wv_eff = merge(w_v, a_v, u_v)

    import ml_dtypes
    BFNP = ml_dtypes.bfloat16
    xT = np.ascontiguousarray(x.reshape(c.seq, c.d).T.astype(BFNP))
    in_maps = []
    for i in range(N_CORES):
        sl = slice(i * c.cpc, (i + 1) * c.cpc)
        in_maps.append({
            "xT": xT,
            "wq": np.ascontiguousarray(wq_eff[:, sl].astype(BFNP)),
            "wk": np.ascontiguousarray(wk_eff[:, sl].astype(BFNP)),
            "wv": np.ascontiguousarray(wv_eff[:, sl].astype(BFNP)),
            "wo": np.ascontiguousarray(w_o[sl, :]),
            "bq": np.ascontiguousarray(b_q[sl]).reshape(c.cpc, 1),
            "bk": np.ascontiguousarray(b_k[sl]).reshape(c.cpc, 1),
        })

    nc = get_nc(cfg)
    res = run_bass_kernel_spmd(nc, in_maps, list(range(N_CORES)))
    out = np.zeros((c.seq, c.d), np.float32)
    for i in range(N_CORES):
        out += res.results[i]["out"]
    # v-bias rides through softmax as a constant row; b_o is plain bias
    out += (b_v @ w_o + b_o).astype(np.float32)
    return out.reshape(B, S, D_MODEL).astype(np.float32)


# revision 45
# speedup vs baseline: 1.0211x; 1.0211x over previous
"""LoRA attention Bass kernel for 8x Trainium2 NeuronCores.

Sharding (Megatron tensor-parallel over heads):
  - Each of the 8 cores owns 2 heads (128 projection columns).
  - q/k/v projections column-sharded; out projection row-sharded;
    per-core partial outputs are summed on the host.
  - LoRA is merged into the base weights on the host (w_eff = w + a@u*scaling),
    exact up to fp32 rounding; host also casts x/wq/wk/wv to bf16.

Device layout (per core), all matmuls bf16 with fp32 PSUM accumulate:
  Phase 1: qT/kT/vT computed transposed ([proj_col, seq]) from xT tiles at
      full PE rate (N=512 moving dim); one batched DMA issue per s-chunk
      brings all 8 k-chunks. PSUM evictions fused with bias + bf16 cast on
      ACT. v natural layout recovered with PE transpose-mode (identity
      matmul), emitted one chunk late so transposes never head-of-line
      block the PE queue. v slabs are 16B-aligned with a fused ones column
      per head ([0:64]=A, 64=onesA, [72:136]=B, 136=onesB).
  Phase 2: S^T = K @ Q^T with BOTH heads as row-tiled concurrent matmuls
      (K=64 at tile_position (0,0)/(64,0)) into one [128,1024] 2-bank PSUM
      tile; one exp over [128,1024] on ACT into bf16. P@V with
      lhsT=[v | ones] so the softmax denominator falls out of the same
      matmul (row 64). PV matmuls are software-pipelined two t-steps behind
      the scores so they never wait on exp at the head of the PE queue.
  Normalization (off the critical path): pv + denom evicted to SBUF right
      after the t-loop (frees PSUM so one pv buffer suffices), denom row
      lane-shifted to partition 0 by a tiny SBUF->SBUF DMA (the custom DVE
      reciprocal and gpsimd partition_broadcast are wrong on HW at nonzero
      base partition), reciprocal_approx_fast, gpsimd partition_broadcast,
      DVE multiply into a [128,512] norm tile whose head-B half is
      lane-shifted by another local DMA.
  Phase 3: out = norm128 @ Wo as single K=128 matmuls (N=512), deferred one
      outer and interleaved into the next outer's t-loop so the PE never
      stalls on the normalization chain; PSUM evicted by DVE, DMAed to DRAM.

PSUM budget (8 banks): scores 2x2 + pv 2 + outproj 2.
"""

import numpy as np

import concourse.bass as bass
import concourse.mybir as mybir
import concourse.tile as tile
from concourse import bacc
from concourse.bass_utils import run_bass_kernel_spmd
from concourse.masks import make_identity
F32 = mybir.dt.float32
F32R = mybir.dt.float32r
BF16 = mybir.dt.bfloat16
AF = mybir.ActivationFunctionType

N_CORES = 8

# Full-problem dims (hardcoded per spec)
D_MODEL = 1024
N_HEADS = 16
D_K = 64
LORA_R = 8
SCALING = 2.0
B = 4
S = 2048


class Cfg:
    """Kernel build configuration."""

    def __init__(self, b=B, s=S, d=D_MODEL, cpc=128, dk=D_K):
        self.b = b                     # batches
        self.s = s                     # seq per batch
        self.d = d                     # model dim (contraction for projections)
        self.cpc = cpc                 # projection cols per core (2 heads x 64)
        self.dk = dk                   # head dim
        self.seq = b * s               # total rows
        self.nkc = d // 128            # k chunks for projections
        self.sc = 512                  # s-chunk width (free dim of matmuls)
        self.nsc = self.seq // self.sc  # s chunks over the whole input
        self.nt = s // 128             # t chunks per batch
        self.nsb = s // self.sc        # s chunks per batch


def _build_nc(cfg: Cfg, dump: bool = False):
    c = cfg
    nc = bacc.Bacc("TRN2", target_bir_lowering=False, debug=False,
                   num_devices=N_CORES)
    n_tchunks_d = c.seq // 128
    if dump:
        qT_d = nc.dram_tensor("qT_d", [128, c.seq], BF16, kind="ExternalOutput").ap()
        kT_d = nc.dram_tensor("kT_d", [128, c.seq], BF16, kind="ExternalOutput").ap()
        v_d = nc.dram_tensor("v_d", [128, n_tchunks_d * 144], BF16, kind="ExternalOutput").ap()
        s_d = nc.dram_tensor("s_d", [128, 1024], F32, kind="ExternalOutput").ap()
        e_d = nc.dram_tensor("e_d", [128, 1024], BF16, kind="ExternalOutput").ap()
        pv_d = nc.dram_tensor("pv_d", [65, 1024], F32, kind="ExternalOutput").ap()
        bcs_d = nc.dram_tensor("bcs_d", [64, 1024], F32, kind="ExternalOutput").ap()
        nrm_d = nc.dram_tensor("nrm_d", [64, 1024], BF16, kind="ExternalOutput").ap()

    xT = nc.dram_tensor("xT", [c.d, c.seq], BF16, kind="ExternalInput").ap()
    wq = nc.dram_tensor("wq", [c.d, c.cpc], BF16, kind="ExternalInput").ap()
    wk = nc.dram_tensor("wk", [c.d, c.cpc], BF16, kind="ExternalInput").ap()
    wv = nc.dram_tensor("wv", [c.d, c.cpc], BF16, kind="ExternalInput").ap()
    wo = nc.dram_tensor("wo", [c.cpc, c.d], F32, kind="ExternalInput").ap()
    bq = nc.dram_tensor("bq", [c.cpc, 1], F32, kind="ExternalInput").ap()
    bk = nc.dram_tensor("bk", [c.cpc, 1], F32, kind="ExternalInput").ap()
    out = nc.dram_tensor("out", [c.seq, c.d], F32, kind="ExternalOutput").ap()

    dk = c.dk
    n_tchunks = c.seq // 128  # global 128-row seq chunks

    with tile.TileContext(nc) as tc:
        with tc.tile_pool(name="persist", bufs=1) as persist:
            # Persistent SBUF tensors
            qT_sb = persist.tile([128, c.seq], BF16, tag="qT")
            kT_sb = persist.tile([128, c.seq], BF16, tag="kT")
            vT_sb = persist.tile([128, c.seq], BF16, tag="vT")
            # v natural + ones columns, 16B-aligned per-head slabs:
            # [0:64]=headA, 64=onesA, [72:136]=headB, 136=onesB, width 144
            VW = 144
            HB = 72
            v_sb = persist.tile([128, n_tchunks, VW], BF16, tag="v")
            wq_sb = persist.tile([128, c.nkc, c.cpc], BF16, tag="wq")
            wk_sb = persist.tile([128, c.nkc, c.cpc], BF16, tag="wk")
            wv_sb = persist.tile([128, c.nkc, c.cpc], BF16, tag="wv")
            wof_sb = persist.tile([c.cpc, c.d], F32, tag="wof")
            wo_sb = persist.tile([c.cpc, c.d], BF16, tag="wo")
            bq_sb = persist.tile([c.cpc, 1], F32, tag="bq")
            bk_sb = persist.tile([c.cpc, 1], F32, tag="bk")

            nc.sync.dma_start(out=wq_sb[:], in_=wq.rearrange("(kc p) m -> p kc m", p=128))
            nc.sync.dma_start(out=wk_sb[:], in_=wk.rearrange("(kc p) m -> p kc m", p=128))
            nc.sync.dma_start(out=wv_sb[:], in_=wv.rearrange("(kc p) m -> p kc m", p=128))
            nc.sync.dma_start(out=wof_sb[:], in_=wo[:])
            nc.sync.dma_start(out=bq_sb[:], in_=bq[:])
            nc.sync.dma_start(out=bk_sb[:], in_=bk[:])
            nc.vector.tensor_copy(wo_sb[:], wof_sb[:])

            ident_sb = persist.tile([128, 128], BF16, tag="ident")
            make_identity(nc, ident_sb[:])

            # ones columns for the fused softmax denominator
            ones_f32 = persist.tile([128, 1], F32, tag="ones_f32")
            nc.vector.memset(ones_f32[:], 1.0)
            nc.vector.tensor_copy(
                v_sb[:, :, dk:dk + 1],
                ones_f32[:].unsqueeze(1).to_broadcast([128, n_tchunks, 1]))
            nc.vector.tensor_copy(
                v_sb[:, :, HB + dk:HB + dk + 1],
                ones_f32[:].unsqueeze(1).to_broadcast([128, n_tchunks, 1]))


            # ---------------- Phase 1: projections ----------------
            with tc.tile_pool(name="xin", bufs=3) as xpool, \
                 tc.tile_pool(name="p1ps", bufs=2, space="PSUM") as p1ps, \
                 tc.tile_pool(name="tpps", bufs=2, space="PSUM") as tpps:
                for sc_i in range(c.nsc):
                    s0 = sc_i * c.sc
                    # one DMA issue brings all 8 k-chunks for this s-range
                    x_t = xpool.tile([128, c.nkc, c.sc], BF16, tag="x")
                    nc.sync.dma_start(
                        out=x_t[:],
                        in_=xT.rearrange("(kc p) s -> p kc s", p=128)[:, :, s0:s0 + c.sc])
                    q_ps = p1ps.tile([128, c.sc], F32, tag="q")
                    k_ps = p1ps.tile([128, c.sc], F32, tag="k")
                    v_ps = p1ps.tile([128, c.sc], F32, tag="v")
                    for kc in range(c.nkc):
                        st = (kc == 0)
                        sp = (kc == c.nkc - 1)
                        nc.tensor.matmul(q_ps[:], wq_sb[:, kc, :], x_t[:, kc, :],
                                         start=st, stop=sp)
                        nc.tensor.matmul(k_ps[:], wk_sb[:, kc, :], x_t[:, kc, :],
                                         start=st, stop=sp)
                        nc.tensor.matmul(v_ps[:], wv_sb[:, kc, :], x_t[:, kc, :],
                                         start=st, stop=sp)
                    nc.scalar.activation(qT_sb[:, s0:s0 + c.sc], q_ps[:],
                                         AF.Identity, bias=bq_sb[:])
                    nc.scalar.activation(kT_sb[:, s0:s0 + c.sc], k_ps[:],
                                         AF.Identity, bias=bk_sb[:])
                    nc.scalar.activation(vT_sb[:, s0:s0 + c.sc], v_ps[:],
                                         AF.Copy)
                    # recover v natural layout with PE transpose-mode, delayed
                    # one chunk so transposes never head-of-line block the PE
                    # queue waiting on this chunk's ACT eviction
                    for sc_t in ([sc_i - 1] if sc_i > 0 else []) + \
                            ([c.nsc - 1] if sc_i == c.nsc - 1 else []):
                        for j in range(c.sc // 128):
                            tci = sc_t * (c.sc // 128) + j
                            t0 = sc_t * c.sc + j * 128
                            tp_ps = tpps.tile([128, 128], BF16, tag="tp")
                            nc.tensor.transpose(
                                tp_ps[:], vT_sb[:, t0:t0 + 128], ident_sb[:])
                            nc.vector.tensor_copy(
                                v_sb[:, tci, 0:dk], tp_ps[:, 0:dk])
                            nc.vector.tensor_copy(
                                v_sb[:, tci, HB:HB + dk], tp_ps[:, dk:2 * dk])

            if dump:
                nc.sync.dma_start(out=qT_d, in_=qT_sb[:])
                nc.sync.dma_start(out=kT_d, in_=kT_sb[:])
                nc.sync.dma_start(out=v_d.rearrange("p (t c) -> p t c", t=n_tchunks_d), in_=v_sb[:])  # width 144

            # ---------------- Phase 2+3: attention + out-proj ----------------
            # PSUM: scores 2x2 banks, pv 2 banks, o_ps 2 banks = 8 banks.
            # pv is evicted to SBUF right after the t-loop so one pv buffer
            # suffices; the normalization chain then runs entirely on SBUF
            # off the critical path, and the out-projection of outer i is
            # interleaved into outer i+1's t-loop (no PE head-of-line stalls).
            with tc.tile_pool(name="sps", bufs=2, space="PSUM") as spool, \
                 tc.tile_pool(name="pvps", bufs=1, space="PSUM") as pvpool, \
                 tc.tile_pool(name="ops", bufs=2, space="PSUM") as opool, \
                 tc.tile_pool(name="exp", bufs=4) as epool, \
                 tc.tile_pool(name="norm", bufs=2) as npool, \
                 tc.tile_pool(name="pvs", bufs=2) as pvspool, \
                 tc.tile_pool(name="bcs", bufs=2) as bcspool, \
                 tc.tile_pool(name="rec", bufs=2) as rpool, \
                 tc.tile_pool(name="osb", bufs=2) as osbpool:
                ew = 512

                def emit_outproj_chunk(norm128, s0, j):
                    o_t = osbpool.tile([128, c.d], F32, tag="osb",
                                       name=f"o_t_{s0}_{j}")
                    for e in range(c.d // ew):
                        o_ps = opool.tile([128, ew], F32, tag="o")
                        nc.tensor.matmul(
                            o_ps[:],
                            norm128[:, j * 128:(j + 1) * 128],
                            wo_sb[:, e * ew:(e + 1) * ew],
                            start=True, stop=True)
                        nc.vector.tensor_copy(
                            o_t[:, e * ew:(e + 1) * ew], o_ps[:])
                    nc.sync.dma_start(
                        out=out[s0 + j * 128:s0 + (j + 1) * 128, :],
                        in_=o_t[:])

                pending = None
                for b_i in range(c.b):
                    for sb_i in range(c.nsb):
                        s0 = b_i * c.s + sb_i * c.sc
                        pv_ab = pvpool.tile([dk + 1, 2 * c.sc], F32, tag="pv")

                        def emit_pv(t, e_ab):
                            tci = b_i * c.nt + t
                            nc.tensor.matmul(
                                pv_ab[:, 0:c.sc], v_sb[:, tci, 0:dk + 1],
                                e_ab[:, 0:c.sc],
                                start=(t == 0), stop=(t == c.nt - 1))
                            nc.tensor.matmul(
                                pv_ab[:, c.sc:2 * c.sc],
                                v_sb[:, tci, HB:HB + dk + 1],
                                e_ab[:, c.sc:2 * c.sc],
                                start=(t == 0), stop=(t == c.nt - 1))

                        # PV is emitted two t-steps late so it never waits on
                        # exp at the head of the PE queue.
                        e_hist = []
                        for t in range(c.nt):
                            t0 = b_i * c.s + t * 128
                            s_ab = spool.tile([128, 2 * c.sc], F32, tag="s")
                            nc.tensor.matmul(
                                s_ab[:, 0:c.sc],
                                kT_sb[0:dk, t0:t0 + 128],
                                qT_sb[0:dk, s0:s0 + c.sc],
                                start=True, stop=True,
                                tile_position=(0, 0))
                            nc.tensor.matmul(
                                s_ab[:, c.sc:2 * c.sc],
                                kT_sb[dk:2 * dk, t0:t0 + 128],
                                qT_sb[dk:2 * dk, s0:s0 + c.sc],
                                start=True, stop=True,
                                tile_position=(64, 0))
                            e_ab = epool.tile([128, 2 * c.sc], BF16, tag="e")
                            nc.scalar.activation(e_ab[:], s_ab[:], AF.Exp,
                                                 scale=1.0 / np.sqrt(dk))
                            if dump and b_i == 0 and sb_i == 0 and t == 0:
                                s_stage = npool.tile([128, 1024], F32, tag="sst")
                                nc.vector.tensor_copy(s_stage[:], s_ab[:])
                                nc.sync.dma_start(out=s_d, in_=s_stage[:])
                                nc.sync.dma_start(out=e_d, in_=e_ab[:])
                            if len(e_hist) == 2:
                                emit_pv(t - 2, e_hist.pop(0))
                            e_hist.append(e_ab)
                            # out-projection of the previous outer, spread
                            # through this t-loop (chain long since done)
                            if pending is not None and t >= 8 and t % 2 == 0:
                                emit_outproj_chunk(pending[0], pending[1],
                                                  (t - 8) // 2)
                        for i, e_h in enumerate(e_hist):
                            emit_pv(c.nt - len(e_hist) + i, e_h)

                        # free PSUM fast: evict pv data + denom row to SBUF
                        den_sb = rpool.tile([dk + 1, 2 * c.sc], F32, tag="den")
                        pvs = pvspool.tile([dk, 2 * c.sc], F32, tag="pvs")
                        nc.vector.tensor_copy(den_sb[dk:dk + 1, :],
                                              pv_ab[dk:dk + 1, :])
                        nc.vector.tensor_copy(pvs[:], pv_ab[0:dk, :])

                        # normalization chain, all on SBUF, off critical path
                        den0 = rpool.tile([1, 2 * c.sc], F32, tag="den0")
                        rec0 = rpool.tile([1, 2 * c.sc], F32, tag="rec0")
                        # lane-shift the denom row to partition 0 (gpsimd
                        # broadcast and the custom recip both need base 0)
                        nc.sync.dma_start(out=den0[0:1, :],
                                          in_=den_sb[dk:dk + 1, :])
                        with nc.allow_low_precision(
                                reason="softmax denom recip at ~18 bits"):
                            nc.vector.reciprocal_approx_fast(
                                out=rec0[0:1, :], in_=den0[0:1, :])
                        bcs_ab = bcspool.tile([dk, 2 * c.sc], F32, tag="bab")
                        nc.gpsimd.partition_broadcast(
                            bcs_ab[:], rec0[0:1, :], channels=dk)
                        norm128 = npool.tile([128, c.sc], BF16, tag="n128")
                        nrm_b = npool.tile([dk, c.sc], BF16, tag="nbt")
                        nc.vector.tensor_tensor(
                            norm128[0:dk, :], pvs[:, 0:c.sc],
                            bcs_ab[:, 0:c.sc], mybir.AluOpType.mult)
                        nc.vector.tensor_tensor(
                            nrm_b[:], pvs[:, c.sc:2 * c.sc],
                            bcs_ab[:, c.sc:2 * c.sc], mybir.AluOpType.mult)
                        # lane-shift head B into partitions 64..127
                        nc.sync.dma_start(out=norm128[dk:2 * dk, :],
                                          in_=nrm_b[:])
                        if dump and b_i == 0 and sb_i == 0:
                            nc.sync.dma_start(out=pv_d[0:dk, :], in_=pvs[:])
                            nc.sync.dma_start(out=bcs_d, in_=bcs_ab[:])
                            nc.sync.dma_start(out=nrm_d[:, 0:512],
                                              in_=norm128[0:dk, :])
                            nc.sync.dma_start(out=nrm_d[:, 512:1024],
                                              in_=nrm_b[:])

                        pending = (norm128, s0)
                if pending is not None:
                    for j in range(c.sc // 128):
                        emit_outproj_chunk(pending[0], pending[1], j)

    nc.compile()
    return nc


_NC_CACHE = {}


def get_nc(cfg: Cfg | None = None):
    cfg = cfg or Cfg()
    key = (cfg.b, cfg.s, cfg.d, cfg.cpc, cfg.dk)
    if key not in _NC_CACHE:
        _NC_CACHE[key] = _build_nc(cfg)
    return _NC_CACHE[key]


def kernel(x, w_q, b_q, w_k, b_k, w_v, b_v, w_o, b_o,
           a_q, u_q, a_k, u_k, a_v, u_v):
    cfg = Cfg()
    c = cfg
    x = np.asarray(x, np.float32)
    w_q = np.asarray(w_q, np.float32)
    w_k = np.asarray(w_k, np.float32)
    w_v = np.asarray(w_v, np.float32)
    w_o = np.asarray(w_o, np.float32)
    b_q = np.asarray(b_q, np.float32)
    b_k = np.asarray(b_k, np.float32)
    b_v = np.asarray(b_v, np.float32)
    b_o = np.asarray(b_o, np.float32)

    def merge(w, a, u):
        return (w.astype(np.float64)
                + (np.asarray(a, np.float64) @ np.asarray(u, np.float64))
                * SCALING).astype(np.float32)

    wq_eff = merge(w_q, a_q, u_q)
    wk_eff = merge(w_k, a_k, u_k)
    wv_eff = merge(w_v, a_v, u_v)

    import ml_dtypes
    BFNP = ml_dtypes.bfloat16
    xT = np.ascontiguousarray(x.reshape(c.seq, c.d).T.astype(BFNP))
    in_maps = []
    for i in range(N_CORES):
        sl = slice(i * c.cpc, (i + 1) * c.cpc)
        in_maps.append({
            "xT": xT,
            "wq": np.ascontiguousarray(wq_eff[:, sl].astype(BFNP)),
            "wk": np.ascontiguousarray(wk_eff[:, sl].astype(BFNP)),
            "wv": np.ascontiguousarray(wv_eff[:, sl].astype(BFNP)),
            "wo": np.ascontiguousarray(w_o[sl, :]),
            "bq": np.ascontiguousarray(b_q[sl]).reshape(c.cpc, 1),
            "bk": np.ascontiguousarray(b_k[sl]).reshape(c.cpc, 1),
        })

    nc = get_nc(cfg)
    res = run_bass_kernel_spmd(nc, in_maps, list(range(N_CORES)))
    out = np.zeros((c.seq, c.d), np.float32)
    for i in range(N_CORES):
        out += res.results[i]["out"]
    # v-bias rides through softmax as a constant row; b_o is plain bias
    out += (b_v @ w_o + b_o).astype(np.float32)
    return out.reshape(B, S, D_MODEL).astype(np.float32)


# revision 46
# speedup vs baseline: 1.0731x; 1.0509x over previous
"""LoRA attention Bass kernel for 8x Trainium2 NeuronCores.

Sharding (Megatron tensor-parallel over heads):
  - Each of the 8 cores owns 2 heads (128 projection columns).
  - q/k/v projections column-sharded; out projection row-sharded;
    per-core partial outputs are summed on the host.
  - LoRA is merged into the base weights on the host (w_eff = w + a@u*scaling),
    exact up to fp32 rounding; host also casts x/wq/wk/wv to bf16.

Device layout (per core), all matmuls bf16 with fp32 PSUM accumulate:
  Phase 1: qT/kT/vT computed transposed ([proj_col, seq]) from xT tiles at
      full PE rate (N=512 moving dim); one batched DMA issue per s-chunk
      brings all 8 k-chunks. PSUM evictions fused with bias + bf16 cast on
      ACT. v natural layout recovered with PE transpose-mode (identity
      matmul), emitted one chunk late so transposes never head-of-line
      block the PE queue. v slabs are 16B-aligned with a fused ones column
      per head ([0:64]=A, 64=onesA, [72:136]=B, 136=onesB).
  Phase 2: S^T = K @ Q^T with BOTH heads as row-tiled concurrent matmuls
      (K=64 at tile_position (0,0)/(64,0)) into one [128,1024] 2-bank PSUM
      tile; one exp over [128,1024] on ACT into bf16. P@V with
      lhsT=[v | ones] so the softmax denominator falls out of the same
      matmul (row 64). PV matmuls are software-pipelined two t-steps behind
      the scores so they never wait on exp at the head of the PE queue.
  Normalization (off the critical path): pv + denom evicted to SBUF right
      after the t-loop (frees PSUM so one pv buffer suffices), denom row
      lane-shifted to partition 0 by a tiny SBUF->SBUF DMA (the custom DVE
      reciprocal and gpsimd partition_broadcast are wrong on HW at nonzero
      base partition), reciprocal_approx_fast, gpsimd partition_broadcast,
      DVE multiply into a [128,512] norm tile whose head-B half is
      lane-shifted by another local DMA.
  Phase 3: out = norm128 @ Wo as single K=128 matmuls (N=512), deferred one
      outer and interleaved into the next outer's t-loop so the PE never
      stalls on the normalization chain; PSUM evicted by DVE, DMAed to DRAM.

PSUM budget (8 banks): scores 2x2 + pv 2 + outproj 2.
"""

import numpy as np

import concourse.bass as bass
import concourse.mybir as mybir
import concourse.tile as tile
from concourse import bacc
from concourse.bass_utils import run_bass_kernel_spmd
from concourse.masks import make_identity
F32 = mybir.dt.float32
F32R = mybir.dt.float32r
BF16 = mybir.dt.bfloat16
AF = mybir.ActivationFunctionType

N_CORES = 8

# Full-problem dims (hardcoded per spec)
D_MODEL = 1024
N_HEADS = 16
D_K = 64
LORA_R = 8
SCALING = 2.0
B = 4
S = 2048


class Cfg:
    """Kernel build configuration."""

    def __init__(self, b=B, s=S, d=D_MODEL, cpc=128, dk=D_K):
        self.b = b                     # batches
        self.s = s                     # seq per batch
        self.d = d                     # model dim (contraction for projections)
        self.cpc = cpc                 # projection cols per core (2 heads x 64)
        self.dk = dk                   # head dim
        self.seq = b * s               # total rows
        self.nkc = d // 128            # k chunks for projections
        self.sc = 512                  # s-chunk width (free dim of matmuls)
        self.nsc = self.seq // self.sc  # s chunks over the whole input
        self.nt = s // 128             # t chunks per batch
        self.nsb = s // self.sc        # s chunks per batch


def _build_nc(cfg: Cfg, dump: bool = False):
    c = cfg
    nc = bacc.Bacc("TRN2", target_bir_lowering=False, debug=False,
                   num_devices=N_CORES)
    n_tchunks_d = c.seq // 128
    if dump:
        qT_d = nc.dram_tensor("qT_d", [128, c.seq], BF16, kind="ExternalOutput").ap()
        kT_d = nc.dram_tensor("kT_d", [128, c.seq], BF16, kind="ExternalOutput").ap()
        v_d = nc.dram_tensor("v_d", [128, n_tchunks_d * 144], BF16, kind="ExternalOutput").ap()
        s_d = nc.dram_tensor("s_d", [128, 1024], F32, kind="ExternalOutput").ap()
        e_d = nc.dram_tensor("e_d", [128, 1024], BF16, kind="ExternalOutput").ap()
        pv_d = nc.dram_tensor("pv_d", [65, 1024], F32, kind="ExternalOutput").ap()
        bcs_d = nc.dram_tensor("bcs_d", [64, 1024], F32, kind="ExternalOutput").ap()
        nrm_d = nc.dram_tensor("nrm_d", [64, 1024], BF16, kind="ExternalOutput").ap()

    xT = nc.dram_tensor("xT", [c.d, c.seq], BF16, kind="ExternalInput").ap()
    wq = nc.dram_tensor("wq", [c.d, c.cpc], BF16, kind="ExternalInput").ap()
    wk = nc.dram_tensor("wk", [c.d, c.cpc], BF16, kind="ExternalInput").ap()
    wv = nc.dram_tensor("wv", [c.d, c.cpc], BF16, kind="ExternalInput").ap()
    wo = nc.dram_tensor("wo", [c.cpc, c.d], F32, kind="ExternalInput").ap()
    bq = nc.dram_tensor("bq", [c.cpc, 1], F32, kind="ExternalInput").ap()
    bk = nc.dram_tensor("bk", [c.cpc, 1], F32, kind="ExternalInput").ap()
    out = nc.dram_tensor("out", [c.seq, c.d], F32, kind="ExternalOutput").ap()

    dk = c.dk
    n_tchunks = c.seq // 128  # global 128-row seq chunks

    with tile.TileContext(nc) as tc:
        with tc.tile_pool(name="persist", bufs=1) as persist:
            # Persistent SBUF tensors
            qT_sb = persist.tile([128, c.seq], BF16, tag="qT")
            kT_sb = persist.tile([128, c.seq], BF16, tag="kT")
            vT_sb = persist.tile([128, c.seq], BF16, tag="vT")
            # v natural + ones columns, 16B-aligned per-head slabs:
            # [0:64]=headA, 64=onesA, [72:136]=headB, 136=onesB, width 144
            VW = 144
            HB = 72
            v_sb = persist.tile([128, n_tchunks, VW], BF16, tag="v")
            wq_sb = persist.tile([128, c.nkc, c.cpc], BF16, tag="wq")
            wk_sb = persist.tile([128, c.nkc, c.cpc], BF16, tag="wk")
            wv_sb = persist.tile([128, c.nkc, c.cpc], BF16, tag="wv")
            wof_sb = persist.tile([c.cpc, c.d], F32, tag="wof")
            wo_sb = persist.tile([c.cpc, c.d], BF16, tag="wo")
            bq_sb = persist.tile([c.cpc, 1], F32, tag="bq")
            bk_sb = persist.tile([c.cpc, 1], F32, tag="bk")

            nc.sync.dma_start(out=wq_sb[:], in_=wq.rearrange("(kc p) m -> p kc m", p=128))
            nc.sync.dma_start(out=wk_sb[:], in_=wk.rearrange("(kc p) m -> p kc m", p=128))
            nc.sync.dma_start(out=wv_sb[:], in_=wv.rearrange("(kc p) m -> p kc m", p=128))
            nc.sync.dma_start(out=wof_sb[:], in_=wo[:])
            nc.sync.dma_start(out=bq_sb[:], in_=bq[:])
            nc.sync.dma_start(out=bk_sb[:], in_=bk[:])
            nc.vector.tensor_copy(wo_sb[:], wof_sb[:])

            ident_sb = persist.tile([128, 128], BF16, tag="ident")
            make_identity(nc, ident_sb[:])

            # ones columns for the fused softmax denominator
            ones_f32 = persist.tile([128, 1], F32, tag="ones_f32")
            nc.vector.memset(ones_f32[:], 1.0)
            nc.vector.tensor_copy(
                v_sb[:, :, dk:dk + 1],
                ones_f32[:].unsqueeze(1).to_broadcast([128, n_tchunks, 1]))
            nc.vector.tensor_copy(
                v_sb[:, :, HB + dk:HB + dk + 1],
                ones_f32[:].unsqueeze(1).to_broadcast([128, n_tchunks, 1]))


            # ---------------- Phase 1: projections ----------------
            with tc.tile_pool(name="xin", bufs=4) as xpool, \
                 tc.tile_pool(name="p1ps", bufs=2, space="PSUM") as p1ps, \
                 tc.tile_pool(name="tpps", bufs=2, space="PSUM") as tpps:
                for sc_i in range(c.nsc):
                    s0 = sc_i * c.sc
                    # one DMA issue brings all 8 k-chunks for this s-range
                    x_t = xpool.tile([128, c.nkc, c.sc], BF16, tag="x")
                    nc.sync.dma_start(
                        out=x_t[:],
                        in_=xT.rearrange("(kc p) s -> p kc s", p=128)[:, :, s0:s0 + c.sc])
                    q_ps = p1ps.tile([128, c.sc], F32, tag="q")
                    k_ps = p1ps.tile([128, c.sc], F32, tag="k")
                    v_ps = p1ps.tile([128, c.sc], F32, tag="v")
                    for kc in range(c.nkc):
                        st = (kc == 0)
                        sp = (kc == c.nkc - 1)
                        nc.tensor.matmul(q_ps[:], wq_sb[:, kc, :], x_t[:, kc, :],
                                         start=st, stop=sp)
                        nc.tensor.matmul(k_ps[:], wk_sb[:, kc, :], x_t[:, kc, :],
                                         start=st, stop=sp)
                        nc.tensor.matmul(v_ps[:], wv_sb[:, kc, :], x_t[:, kc, :],
                                         start=st, stop=sp)
                    nc.scalar.activation(qT_sb[:, s0:s0 + c.sc], q_ps[:],
                                         AF.Identity, bias=bq_sb[:])
                    nc.scalar.activation(kT_sb[:, s0:s0 + c.sc], k_ps[:],
                                         AF.Identity, bias=bk_sb[:])
                    nc.scalar.activation(vT_sb[:, s0:s0 + c.sc], v_ps[:],
                                         AF.Copy)
                    # recover v natural layout with PE transpose-mode, delayed
                    # one chunk so transposes never head-of-line block the PE
                    # queue waiting on this chunk's ACT eviction
                    for sc_t in ([sc_i - 1] if sc_i > 0 else []) + \
                            ([c.nsc - 1] if sc_i == c.nsc - 1 else []):
                        for j in range(c.sc // 128):
                            tci = sc_t * (c.sc // 128) + j
                            t0 = sc_t * c.sc + j * 128
                            tp_ps = tpps.tile([128, 128], BF16, tag="tp")
                            nc.tensor.transpose(
                                tp_ps[:], vT_sb[:, t0:t0 + 128], ident_sb[:])
                            nc.vector.tensor_copy(
                                v_sb[:, tci, 0:dk], tp_ps[:, 0:dk])
                            nc.vector.tensor_copy(
                                v_sb[:, tci, HB:HB + dk], tp_ps[:, dk:2 * dk])

            if dump:
                nc.sync.dma_start(out=qT_d, in_=qT_sb[:])
                nc.sync.dma_start(out=kT_d, in_=kT_sb[:])
                nc.sync.dma_start(out=v_d.rearrange("p (t c) -> p t c", t=n_tchunks_d), in_=v_sb[:])  # width 144

            # ---------------- Phase 2+3: attention + out-proj ----------------
            # PSUM: scores 2x2 banks, pv 2 banks, o_ps 2 banks = 8 banks.
            # pv is evicted to SBUF right after the t-loop so one pv buffer
            # suffices; the normalization chain then runs entirely on SBUF
            # off the critical path, and the out-projection of outer i is
            # interleaved into outer i+1's t-loop (no PE head-of-line stalls).
            with tc.tile_pool(name="sps", bufs=2, space="PSUM") as spool, \
                 tc.tile_pool(name="pvps", bufs=1, space="PSUM") as pvpool, \
                 tc.tile_pool(name="ops", bufs=2, space="PSUM") as opool, \
                 tc.tile_pool(name="exp", bufs=6) as epool, \
                 tc.tile_pool(name="norm", bufs=3) as npool, \
                 tc.tile_pool(name="pvs", bufs=3) as pvspool, \
                 tc.tile_pool(name="bcs", bufs=3) as bcspool, \
                 tc.tile_pool(name="rec", bufs=3) as rpool, \
                 tc.tile_pool(name="osb", bufs=3) as osbpool:
                ew = 512

                def emit_outproj_chunk(norm128, s0, j):
                    o_t = osbpool.tile([128, c.d], F32, tag="osb",
                                       name=f"o_t_{s0}_{j}")
                    for e in range(c.d // ew):
                        o_ps = opool.tile([128, ew], F32, tag="o")
                        nc.tensor.matmul(
                            o_ps[:],
                            norm128[:, j * 128:(j + 1) * 128],
                            wo_sb[:, e * ew:(e + 1) * ew],
                            start=True, stop=True)
                        nc.vector.tensor_copy(
                            o_t[:, e * ew:(e + 1) * ew], o_ps[:])
                    nc.sync.dma_start(
                        out=out[s0 + j * 128:s0 + (j + 1) * 128, :],
                        in_=o_t[:])

                pending = None
                for b_i in range(c.b):
                    for sb_i in range(c.nsb):
                        s0 = b_i * c.s + sb_i * c.sc
                        pv_ab = pvpool.tile([dk + 1, 2 * c.sc], F32, tag="pv")

                        def emit_pv(t, e_ab):
                            tci = b_i * c.nt + t
                            nc.tensor.matmul(
                                pv_ab[:, 0:c.sc], v_sb[:, tci, 0:dk + 1],
                                e_ab[:, 0:c.sc],
                                start=(t == 0), stop=(t == c.nt - 1))
                            nc.tensor.matmul(
                                pv_ab[:, c.sc:2 * c.sc],
                                v_sb[:, tci, HB:HB + dk + 1],
                                e_ab[:, c.sc:2 * c.sc],
                                start=(t == 0), stop=(t == c.nt - 1))

                        # PV is emitted two t-steps late so it never waits on
                        # exp at the head of the PE queue.
                        e_hist = []
                        for t in range(c.nt):
                            t0 = b_i * c.s + t * 128
                            s_ab = spool.tile([128, 2 * c.sc], F32, tag="s")
                            nc.tensor.matmul(
                                s_ab[:, 0:c.sc],
                                kT_sb[0:dk, t0:t0 + 128],
                                qT_sb[0:dk, s0:s0 + c.sc],
                                start=True, stop=True,
                                tile_position=(0, 0))
                            nc.tensor.matmul(
                                s_ab[:, c.sc:2 * c.sc],
                                kT_sb[dk:2 * dk, t0:t0 + 128],
                                qT_sb[dk:2 * dk, s0:s0 + c.sc],
                                start=True, stop=True,
                                tile_position=(64, 0))
                            e_ab = epool.tile([128, 2 * c.sc], BF16, tag="e")
                            nc.scalar.activation(e_ab[:], s_ab[:], AF.Exp,
                                                 scale=1.0 / np.sqrt(dk))
                            if dump and b_i == 0 and sb_i == 0 and t == 0:
                                s_stage = npool.tile([128, 1024], F32, tag="sst")
                                nc.vector.tensor_copy(s_stage[:], s_ab[:])
                                nc.sync.dma_start(out=s_d, in_=s_stage[:])
                                nc.sync.dma_start(out=e_d, in_=e_ab[:])
                            if len(e_hist) == 2:
                                emit_pv(t - 2, e_hist.pop(0))
                            e_hist.append(e_ab)
                            # out-projection of the previous outer, spread
                            # through this t-loop (chain long since done)
                            if pending is not None and t >= 8 and t % 2 == 0:
                                emit_outproj_chunk(pending[0], pending[1],
                                                  (t - 8) // 2)
                        for i, e_h in enumerate(e_hist):
                            emit_pv(c.nt - len(e_hist) + i, e_h)

                        # free PSUM fast: evict pv data + denom row to SBUF
                        den_sb = rpool.tile([dk + 1, 2 * c.sc], F32, tag="den")
                        pvs = pvspool.tile([dk, 2 * c.sc], F32, tag="pvs")
                        nc.vector.tensor_copy(den_sb[dk:dk + 1, :],
                                              pv_ab[dk:dk + 1, :])
                        nc.vector.tensor_copy(pvs[:], pv_ab[0:dk, :])

                        # normalization chain, all on SBUF, off critical path
                        den0 = rpool.tile([1, 2 * c.sc], F32, tag="den0")
                        rec0 = rpool.tile([1, 2 * c.sc], F32, tag="rec0")
                        # lane-shift the denom row to partition 0 (gpsimd
                        # broadcast and the custom recip both need base 0)
                        nc.sync.dma_start(out=den0[0:1, :],
                                          in_=den_sb[dk:dk + 1, :])
                        with nc.allow_low_precision(
                                reason="softmax denom recip at ~18 bits"):
                            nc.vector.reciprocal_approx_fast(
                                out=rec0[0:1, :], in_=den0[0:1, :])
                        bcs_ab = bcspool.tile([dk, 2 * c.sc], F32, tag="bab")
                        nc.gpsimd.partition_broadcast(
                            bcs_ab[:], rec0[0:1, :], channels=dk)
                        norm128 = npool.tile([128, c.sc], BF16, tag="n128")
                        nrm_b = npool.tile([dk, c.sc], BF16, tag="nbt")
                        nc.vector.tensor_tensor(
                            norm128[0:dk, :], pvs[:, 0:c.sc],
                            bcs_ab[:, 0:c.sc], mybir.AluOpType.mult)
                        nc.vector.tensor_tensor(
                            nrm_b[:], pvs[:, c.sc:2 * c.sc],
                            bcs_ab[:, c.sc:2 * c.sc], mybir.AluOpType.mult)
                        # lane-shift head B into partitions 64..127
                        nc.sync.dma_start(out=norm128[dk:2 * dk, :],
                                          in_=nrm_b[:])
                        if dump and b_i == 0 and sb_i == 0:
                            nc.sync.dma_start(out=pv_d[0:dk, :], in_=pvs[:])
                            nc.sync.dma_start(out=bcs_d, in_=bcs_ab[:])
                            nc.sync.dma_start(out=nrm_d[:, 0:512],
                                              in_=norm128[0:dk, :])
                            nc.sync.dma_start(out=nrm_d[:, 512:1024],
                                              in_=nrm_b[:])

                        pending = (norm128, s0)
                if pending is not None:
                    for j in range(c.sc // 128):
                        emit_outproj_chunk(pending[0], pending[1], j)

    nc.compile()
    return nc


_NC_CACHE = {}


def get_nc(cfg: Cfg | None = None):
    cfg = cfg or Cfg()
    key = (cfg.b, cfg.s, cfg.d, cfg.cpc, cfg.dk)
    if key not in _NC_CACHE:
        _NC_CACHE[key] = _build_nc(cfg)
    return _NC_CACHE[key]


def kernel(x, w_q, b_q, w_k, b_k, w_v, b_v, w_o, b_o,
           a_q, u_q, a_k, u_k, a_v, u_v):
    cfg = Cfg()
    c = cfg
    x = np.asarray(x, np.float32)
    w_q = np.asarray(w_q, np.float32)
    w_k = np.asarray(w_k, np.float32)
    w_v = np.asarray(w_v, np.float32)
    w_o = np.asarray(w_o, np.float32)
    b_q = np.asarray(b_q, np.float32)
    b_k = np.asarray(b_k, np.float32)
    b_v = np.asarray(b_v, np.float32)
    b_o = np.asarray(b_o, np.float32)

    def merge(w, a, u):
        return (w.astype(np.float64)
                + (np.asarray(a, np.float64) @ np.asarray(u, np.float64))
                * SCALING).astype(np.float32)

    wq_eff = merge(w_q, a_q, u_q)
    wk_eff = merge(w_k, a_k, u_k)
    wv_eff = merge(w_v, a_v, u_v)

    import ml_dtypes
    BFNP = ml_dtypes.bfloat16
    xT = np.ascontiguousarray(x.reshape(c.seq, c.d).T.astype(BFNP))
    in_maps = []
    for i in range(N_CORES):
        sl = slice(i * c.cpc, (i + 1) * c.cpc)
        in_maps.append({
            "xT": xT,
            "wq": np.ascontiguousarray(wq_eff[:, sl].astype(BFNP)),
            "wk": np.ascontiguousarray(wk_eff[:, sl].astype(BFNP)),
            "wv": np.ascontiguousarray(wv_eff[:, sl].astype(BFNP)),
            "wo": np.ascontiguousarray(w_o[sl, :]),
            "bq": np.ascontiguousarray(b_q[sl]).reshape(c.cpc, 1),
            "bk": np.ascontiguousarray(b_k[sl]).reshape(c.cpc, 1),
        })

    nc = get_nc(cfg)
    res = run_bass_kernel_spmd(nc, in_maps, list(range(N_CORES)))
    out = np.zeros((c.seq, c.d), np.float32)
    for i in range(N_CORES):
        out += res.results[i]["out"]
    # v-bias rides through softmax as a constant row; b_o is plain bias
    out += (b_v @ w_o + b_o).astype(np.float32)
    return out.reshape(B, S, D_MODEL).astype(np.float32)


# revision 47
# speedup vs baseline: 1.0834x; 1.0096x over previous
"""LoRA attention Bass kernel for 8x Trainium2 NeuronCores.

Sharding (Megatron tensor-parallel over heads):
  - Each of the 8 cores owns 2 heads (128 projection columns).
  - q/k/v projections column-sharded; out projection row-sharded;
    per-core partial outputs are summed on the host.
  - LoRA is merged into the base weights on the host (w_eff = w + a@u*scaling),
    exact up to fp32 rounding; host also casts x/wq/wk/wv to bf16.

Device layout (per core), all matmuls bf16 with fp32 PSUM accumulate:
  Phase 1: qT/kT/vT computed transposed ([proj_col, seq]) from xT tiles at
      full PE rate (N=512 moving dim); one batched DMA issue per s-chunk
      brings all 8 k-chunks. PSUM evictions fused with bias + bf16 cast on
      ACT. v natural layout recovered with PE transpose-mode (identity
      matmul), emitted one chunk late so transposes never head-of-line
      block the PE queue. v slabs are 16B-aligned with a fused ones column
      per head ([0:64]=A, 64=onesA, [72:136]=B, 136=onesB).
  Phase 2: S^T = K @ Q^T with BOTH heads as row-tiled concurrent matmuls
      (K=64 at tile_position (0,0)/(64,0)) into one [128,1024] 2-bank PSUM
      tile; one exp over [128,1024] on ACT into bf16. P@V with
      lhsT=[v | ones] so the softmax denominator falls out of the same
      matmul (row 64). PV matmuls are software-pipelined two t-steps behind
      the scores so they never wait on exp at the head of the PE queue.
  Normalization (off the critical path): pv + denom evicted to SBUF right
      after the t-loop (frees PSUM so one pv buffer suffices), denom row
      lane-shifted to partition 0 by a tiny SBUF->SBUF DMA (the custom DVE
      reciprocal and gpsimd partition_broadcast are wrong on HW at nonzero
      base partition), reciprocal_approx_fast, gpsimd partition_broadcast,
      DVE multiply into a [128,512] norm tile whose head-B half is
      lane-shifted by another local DMA.
  Phase 3: out = norm128 @ Wo as single K=128 matmuls (N=512), deferred one
      outer and interleaved into the next outer's t-loop so the PE never
      stalls on the normalization chain; PSUM evicted by DVE, DMAed to DRAM.

PSUM budget (8 banks): scores 2x2 + pv 2 + outproj 2.
"""

import numpy as np

import concourse.bass as bass
import concourse.mybir as mybir
import concourse.tile as tile
from concourse import bacc
from concourse.bass_utils import run_bass_kernel_spmd
from concourse.masks import make_identity
F32 = mybir.dt.float32
F32R = mybir.dt.float32r
BF16 = mybir.dt.bfloat16
AF = mybir.ActivationFunctionType

N_CORES = 8

# Full-problem dims (hardcoded per spec)
D_MODEL = 1024
N_HEADS = 16
D_K = 64
LORA_R = 8
SCALING = 2.0
B = 4
S = 2048


class Cfg:
    """Kernel build configuration."""

    def __init__(self, b=B, s=S, d=D_MODEL, cpc=128, dk=D_K):
        self.b = b                     # batches
        self.s = s                     # seq per batch
        self.d = d                     # model dim (contraction for projections)
        self.cpc = cpc                 # projection cols per core (2 heads x 64)
        self.dk = dk                   # head dim
        self.seq = b * s               # total rows
        self.nkc = d // 128            # k chunks for projections
        self.sc = 512                  # s-chunk width (free dim of matmuls)
        self.nsc = self.seq // self.sc  # s chunks over the whole input
        self.nt = s // 128             # t chunks per batch
        self.nsb = s // self.sc        # s chunks per batch


def _build_nc(cfg: Cfg, dump: bool = False):
    c = cfg
    nc = bacc.Bacc("TRN2", target_bir_lowering=False, debug=False,
                   num_devices=N_CORES)
    n_tchunks_d = c.seq // 128
    if dump:
        qT_d = nc.dram_tensor("qT_d", [128, c.seq], BF16, kind="ExternalOutput").ap()
        kT_d = nc.dram_tensor("kT_d", [128, c.seq], BF16, kind="ExternalOutput").ap()
        v_d = nc.dram_tensor("v_d", [128, n_tchunks_d * 144], BF16, kind="ExternalOutput").ap()
        s_d = nc.dram_tensor("s_d", [128, 1024], F32, kind="ExternalOutput").ap()
        e_d = nc.dram_tensor("e_d", [128, 1024], BF16, kind="ExternalOutput").ap()
        pv_d = nc.dram_tensor("pv_d", [65, 1024], F32, kind="ExternalOutput").ap()
        bcs_d = nc.dram_tensor("bcs_d", [64, 1024], F32, kind="ExternalOutput").ap()
        nrm_d = nc.dram_tensor("nrm_d", [64, 1024], BF16, kind="ExternalOutput").ap()

    xT = nc.dram_tensor("xT", [c.d, c.seq], BF16, kind="ExternalInput").ap()
    wq = nc.dram_tensor("wq", [c.d, c.cpc], BF16, kind="ExternalInput").ap()
    wk = nc.dram_tensor("wk", [c.d, c.cpc], BF16, kind="ExternalInput").ap()
    wv = nc.dram_tensor("wv", [c.d, c.cpc], BF16, kind="ExternalInput").ap()
    wo = nc.dram_tensor("wo", [c.cpc, c.d], F32, kind="ExternalInput").ap()
    bq = nc.dram_tensor("bq", [c.cpc, 1], F32, kind="ExternalInput").ap()
    bk = nc.dram_tensor("bk", [c.cpc, 1], F32, kind="ExternalInput").ap()
    out = nc.dram_tensor("out", [c.seq, c.d], F32, kind="ExternalOutput").ap()

    dk = c.dk
    n_tchunks = c.seq // 128  # global 128-row seq chunks

    with tile.TileContext(nc) as tc:
        with tc.tile_pool(name="persist", bufs=1) as persist:
            # Persistent SBUF tensors
            qT_sb = persist.tile([128, c.seq], BF16, tag="qT")
            kT_sb = persist.tile([128, c.seq], BF16, tag="kT")
            vT_sb = persist.tile([128, c.seq], BF16, tag="vT")
            # v natural + ones columns, 16B-aligned per-head slabs:
            # [0:64]=headA, 64=onesA, [72:136]=headB, 136=onesB, width 144
            VW = 144
            HB = 72
            v_sb = persist.tile([128, n_tchunks, VW], BF16, tag="v")
            wq_sb = persist.tile([128, c.nkc, c.cpc], BF16, tag="wq")
            wk_sb = persist.tile([128, c.nkc, c.cpc], BF16, tag="wk")
            wv_sb = persist.tile([128, c.nkc, c.cpc], BF16, tag="wv")
            wof_sb = persist.tile([c.cpc, c.d], F32, tag="wof")
            wo_sb = persist.tile([c.cpc, c.d], BF16, tag="wo")
            bq_sb = persist.tile([c.cpc, 1], F32, tag="bq")
            bk_sb = persist.tile([c.cpc, 1], F32, tag="bk")

            nc.sync.dma_start(out=wq_sb[:], in_=wq.rearrange("(kc p) m -> p kc m", p=128))
            nc.sync.dma_start(out=wk_sb[:], in_=wk.rearrange("(kc p) m -> p kc m", p=128))
            nc.sync.dma_start(out=wv_sb[:], in_=wv.rearrange("(kc p) m -> p kc m", p=128))
            nc.sync.dma_start(out=wof_sb[:], in_=wo[:])
            nc.sync.dma_start(out=bq_sb[:], in_=bq[:])
            nc.sync.dma_start(out=bk_sb[:], in_=bk[:])
            nc.vector.tensor_copy(wo_sb[:], wof_sb[:])

            ident_sb = persist.tile([128, 128], BF16, tag="ident")
            make_identity(nc, ident_sb[:])

            # ones columns for the fused softmax denominator
            ones_f32 = persist.tile([128, 1], F32, tag="ones_f32")
            nc.vector.memset(ones_f32[:], 1.0)
            nc.vector.tensor_copy(
                v_sb[:, :, dk:dk + 1],
                ones_f32[:].unsqueeze(1).to_broadcast([128, n_tchunks, 1]))
            nc.vector.tensor_copy(
                v_sb[:, :, HB + dk:HB + dk + 1],
                ones_f32[:].unsqueeze(1).to_broadcast([128, n_tchunks, 1]))


            # ---------------- Phase 1: projections ----------------
            with tc.tile_pool(name="xin", bufs=4) as xpool, \
                 tc.tile_pool(name="p1ps", bufs=2, space="PSUM") as p1ps, \
                 tc.tile_pool(name="tpps", bufs=2, space="PSUM") as tpps:
                for sc_i in range(c.nsc):
                    s0 = sc_i * c.sc
                    # one DMA issue brings all 8 k-chunks for this s-range
                    x_t = xpool.tile([128, c.nkc, c.sc], BF16, tag="x")
                    nc.sync.dma_start(
                        out=x_t[:],
                        in_=xT.rearrange("(kc p) s -> p kc s", p=128)[:, :, s0:s0 + c.sc])
                    q_ps = p1ps.tile([128, c.sc], F32, tag="q")
                    k_ps = p1ps.tile([128, c.sc], F32, tag="k")
                    v_ps = p1ps.tile([128, c.sc], F32, tag="v")
                    for kc in range(c.nkc):
                        st = (kc == 0)
                        sp = (kc == c.nkc - 1)
                        nc.tensor.matmul(q_ps[:], wq_sb[:, kc, :], x_t[:, kc, :],
                                         start=st, stop=sp)
                        nc.tensor.matmul(k_ps[:], wk_sb[:, kc, :], x_t[:, kc, :],
                                         start=st, stop=sp)
                        nc.tensor.matmul(v_ps[:], wv_sb[:, kc, :], x_t[:, kc, :],
                                         start=st, stop=sp)
                    nc.scalar.activation(qT_sb[:, s0:s0 + c.sc], q_ps[:],
                                         AF.Identity, bias=bq_sb[:])
                    nc.scalar.activation(kT_sb[:, s0:s0 + c.sc], k_ps[:],
                                         AF.Identity, bias=bk_sb[:])
                    nc.scalar.activation(vT_sb[:, s0:s0 + c.sc], v_ps[:],
                                         AF.Copy)
                    # recover v natural layout with PE transpose-mode, delayed
                    # one chunk so transposes never head-of-line block the PE
                    # queue waiting on this chunk's ACT eviction
                    for sc_t in ([sc_i - 1] if sc_i > 0 else []) + \
                            ([c.nsc - 1] if sc_i == c.nsc - 1 else []):
                        for j in range(c.sc // 128):
                            tci = sc_t * (c.sc // 128) + j
                            t0 = sc_t * c.sc + j * 128
                            tp_ps = tpps.tile([128, 128], BF16, tag="tp")
                            nc.tensor.transpose(
                                tp_ps[:], vT_sb[:, t0:t0 + 128], ident_sb[:])
                            nc.vector.tensor_copy(
                                v_sb[:, tci, 0:dk], tp_ps[:, 0:dk])
                            nc.vector.tensor_copy(
                                v_sb[:, tci, HB:HB + dk], tp_ps[:, dk:2 * dk])

            if dump:
                nc.sync.dma_start(out=qT_d, in_=qT_sb[:])
                nc.sync.dma_start(out=kT_d, in_=kT_sb[:])
                nc.sync.dma_start(out=v_d.rearrange("p (t c) -> p t c", t=n_tchunks_d), in_=v_sb[:])  # width 144

            # ---------------- Phase 2+3: attention + out-proj ----------------
            # PSUM: scores 2x2 banks, pv 2 banks, o_ps 2 banks = 8 banks.
            # pv is evicted to SBUF right after the t-loop so one pv buffer
            # suffices; the normalization chain then runs entirely on SBUF
            # off the critical path, and the out-projection of outer i is
            # interleaved into outer i+1's t-loop (no PE head-of-line stalls).
            with tc.tile_pool(name="sps", bufs=2, space="PSUM") as spool, \
                 tc.tile_pool(name="pvps", bufs=1, space="PSUM") as pvpool, \
                 tc.tile_pool(name="ops", bufs=2, space="PSUM") as opool, \
                 tc.tile_pool(name="exp", bufs=8) as epool, \
                 tc.tile_pool(name="norm", bufs=3) as npool, \
                 tc.tile_pool(name="pvs", bufs=3) as pvspool, \
                 tc.tile_pool(name="bcs", bufs=3) as bcspool, \
                 tc.tile_pool(name="rec", bufs=3) as rpool, \
                 tc.tile_pool(name="osb", bufs=3) as osbpool:
                ew = 512

                def emit_outproj_chunk(norm128, s0, j):
                    o_t = osbpool.tile([128, c.d], F32, tag="osb",
                                       name=f"o_t_{s0}_{j}")
                    for e in range(c.d // ew):
                        o_ps = opool.tile([128, ew], F32, tag="o")
                        nc.tensor.matmul(
                            o_ps[:],
                            norm128[:, j * 128:(j + 1) * 128],
                            wo_sb[:, e * ew:(e + 1) * ew],
                            start=True, stop=True)
                        nc.vector.tensor_copy(
                            o_t[:, e * ew:(e + 1) * ew], o_ps[:])
                    nc.sync.dma_start(
                        out=out[s0 + j * 128:s0 + (j + 1) * 128, :],
                        in_=o_t[:])

                pending = None
                for b_i in range(c.b):
                    for sb_i in range(c.nsb):
                        s0 = b_i * c.s + sb_i * c.sc
                        pv_ab = pvpool.tile([dk + 1, 2 * c.sc], F32, tag="pv")

                        def emit_pv(t, e_ab):
                            tci = b_i * c.nt + t
                            nc.tensor.matmul(
                                pv_ab[:, 0:c.sc], v_sb[:, tci, 0:dk + 1],
                                e_ab[:, 0:c.sc],
                                start=(t == 0), stop=(t == c.nt - 1))
                            nc.tensor.matmul(
                                pv_ab[:, c.sc:2 * c.sc],
                                v_sb[:, tci, HB:HB + dk + 1],
                                e_ab[:, c.sc:2 * c.sc],
                                start=(t == 0), stop=(t == c.nt - 1))

                        # PV is emitted two t-steps late so it never waits on
                        # exp at the head of the PE queue.
                        e_hist = []
                        for t in range(c.nt):
                            t0 = b_i * c.s + t * 128
                            s_ab = spool.tile([128, 2 * c.sc], F32, tag="s")
                            nc.tensor.matmul(
                                s_ab[:, 0:c.sc],
                                kT_sb[0:dk, t0:t0 + 128],
                                qT_sb[0:dk, s0:s0 + c.sc],
                                start=True, stop=True,
                                tile_position=(0, 0))
                            nc.tensor.matmul(
                                s_ab[:, c.sc:2 * c.sc],
                                kT_sb[dk:2 * dk, t0:t0 + 128],
                                qT_sb[dk:2 * dk, s0:s0 + c.sc],
                                start=True, stop=True,
                                tile_position=(64, 0))
                            e_ab = epool.tile([128, 2 * c.sc], BF16, tag="e")
                            nc.scalar.activation(e_ab[:], s_ab[:], AF.Exp,
                                                 scale=1.0 / np.sqrt(dk))
                            if dump and b_i == 0 and sb_i == 0 and t == 0:
                                s_stage = npool.tile([128, 1024], F32, tag="sst")
                                nc.vector.tensor_copy(s_stage[:], s_ab[:])
                                nc.sync.dma_start(out=s_d, in_=s_stage[:])
                                nc.sync.dma_start(out=e_d, in_=e_ab[:])
                            if len(e_hist) == 2:
                                emit_pv(t - 2, e_hist.pop(0))
                            e_hist.append(e_ab)
                            # out-projection of the previous outer, spread
                            # through this t-loop (chain long since done)
                            if pending is not None and t >= 8 and t % 2 == 0:
                                emit_outproj_chunk(pending[0], pending[1],
                                                  (t - 8) // 2)
                        for i, e_h in enumerate(e_hist):
                            emit_pv(c.nt - len(e_hist) + i, e_h)

                        # free PSUM fast: evict pv data + denom row to SBUF
                        den_sb = rpool.tile([dk + 1, 2 * c.sc], F32, tag="den")
                        pvs = pvspool.tile([dk, 2 * c.sc], F32, tag="pvs")
                        nc.vector.tensor_copy(den_sb[dk:dk + 1, :],
                                              pv_ab[dk:dk + 1, :])
                        nc.vector.tensor_copy(pvs[:], pv_ab[0:dk, :])

                        # normalization chain, all on SBUF, off critical path
                        den0 = rpool.tile([1, 2 * c.sc], F32, tag="den0")
                        rec0 = rpool.tile([1, 2 * c.sc], F32, tag="rec0")
                        # lane-shift the denom row to partition 0 (gpsimd
                        # broadcast and the custom recip both need base 0)
                        nc.sync.dma_start(out=den0[0:1, :],
                                          in_=den_sb[dk:dk + 1, :])
                        with nc.allow_low_precision(
                                reason="softmax denom recip at ~18 bits"):
                            nc.vector.reciprocal_approx_fast(
                                out=rec0[0:1, :], in_=den0[0:1, :])
                        bcs_ab = bcspool.tile([dk, 2 * c.sc], F32, tag="bab")
                        nc.gpsimd.partition_broadcast(
                            bcs_ab[:], rec0[0:1, :], channels=dk)
                        norm128 = npool.tile([128, c.sc], BF16, tag="n128")
                        nrm_b = npool.tile([dk, c.sc], BF16, tag="nbt")
                        nc.vector.tensor_tensor(
                            norm128[0:dk, :], pvs[:, 0:c.sc],
                            bcs_ab[:, 0:c.sc], mybir.AluOpType.mult)
                        nc.vector.tensor_tensor(
                            nrm_b[:], pvs[:, c.sc:2 * c.sc],
                            bcs_ab[:, c.sc:2 * c.sc], mybir.AluOpType.mult)
                        # lane-shift head B into partitions 64..127
                        nc.sync.dma_start(out=norm128[dk:2 * dk, :],
                                          in_=nrm_b[:])
                        if dump and b_i == 0 and sb_i == 0:
                            nc.sync.dma_start(out=pv_d[0:dk, :], in_=pvs[:])
                            nc.sync.dma_start(out=bcs_d, in_=bcs_ab[:])
                            nc.sync.dma_start(out=nrm_d[:, 0:512],
                                              in_=norm128[0:dk, :])
                            nc.sync.dma_start(out=nrm_d[:, 512:1024],
                                              in_=nrm_b[:])

                        pending = (norm128, s0)
                if pending is not None:
                    for j in range(c.sc // 128):
                        emit_outproj_chunk(pending[0], pending[1], j)

    nc.compile()
    return nc


_NC_CACHE = {}


def get_nc(cfg: Cfg | None = None):
    cfg = cfg or Cfg()
    key = (cfg.b, cfg.s, cfg.d, cfg.cpc, cfg.dk)
    if key not in _NC_CACHE:
        _NC_CACHE[key] = _build_nc(cfg)
    return _NC_CACHE[key]


def kernel(x, w_q, b_q, w_k, b_k, w_v, b_v, w_o, b_o,
           a_q, u_q, a_k, u_k, a_v, u_v):
    cfg = Cfg()
    c = cfg
    x = np.asarray(x, np.float32)
    w_q = np.asarray(w_q, np.float32)
    w_k = np.asarray(w_k, np.float32)
    w_v = np.asarray(w_v, np.float32)
    w_o = np.asarray(w_o, np.float32)
    b_q = np.asarray(b_q, np.float32)
    b_k = np.asarray(b_k, np.float32)
    b_v = np.asarray(b_v, np.float32)
    b_o = np.asarray(b_o, np.float32)

    def merge(w, a, u):
        return (w.astype(np.float64)
                + (np.asarray(a, np.float64) @ np.asarray(u, np.float64))
                * SCALING).astype(np.float32)

    wq_eff = merge(w_q, a_q, u_q)
    wk_eff = merge(w_k, a_k, u_k)
    wv_eff = merge(w_v, a_v, u_v)

    import ml_dtypes
    BFNP = ml_dtypes.bfloat16
    xT = np.ascontiguousarray(x.reshape(c.seq, c.d).T.astype(BFNP))
    in_maps = []
    for i in range(N_CORES):
        sl = slice(i * c.cpc, (i + 1) * c.cpc)
        in_maps.append({
            "xT": xT,
            "wq": np.ascontiguousarray(wq_eff[:, sl].astype(BFNP)),
            "wk": np.ascontiguousarray(wk_eff[:, sl].astype(BFNP)),
            "wv": np.ascontiguousarray(wv_eff[:, sl].astype(BFNP)),
            "wo": np.ascontiguousarray(w_o[sl, :]),
            "bq": np.ascontiguousarray(b_q[sl]).reshape(c.cpc, 1),
            "bk": np.ascontiguousarray(b_k[sl]).reshape(c.cpc, 1),
        })

    nc = get_nc(cfg)
    res = run_bass_kernel_spmd(nc, in_maps, list(range(N_CORES)))
    out = np.zeros((c.seq, c.d), np.float32)
    for i in range(N_CORES):
        out += res.results[i]["out"]
    # v-bias rides through softmax as a constant row; b_o is plain bias
    out += (b_v @ w_o + b_o).astype(np.float32)
    return out.reshape(B, S, D_MODEL).astype(np.float32)
